# revision 22
# baseline (speedup 1.0000x reference)
"""Causal attention (QKV proj + softmax + PV + ReLU) on 8 trn2 NeuronCores.

Sharding: data-parallel over batch B=32 -> 4 batches per core; projection
weights replicated.

Dtypes: everything that can tolerate it runs in fp8-e4m3 with DoubleRow
matmuls (two 128-row contraction slabs per pass -> 2x PE throughput vs
fp16): the Q/K projections, S=K^T.T@Q^T, the V projection, and P@V.
Accumulation is always fp32 in PSUM. Error control:
  - weights are pre-scaled by 16 on the host so their U(-1/32,1/32)
    entries escape e4m3's subnormal range (descale folded into drains; for
    V the 16x rides the value domain and cancels exactly in the rowsum
    normalization, via 16.0-valued `ones`).
  - rows i<128 get a full fp16 path (fp16 projections of q,k,v for l<128,
    fp16 S block, fp16 PV for ib=0): few-term softmax rows pass quant
    noise straight to the output, everything later averages it away.
    Measured rel-err 6.7e-3 vs the 2e-2 gate.
  - the fp16 early-block q,k come from V-style x-stationary N=512 matmuls
    (full PE rate) + 8 PE transposes to d-major, interleaved into the V
    phase so transpose LDWEIGHTS hide under N=512 streams.

Schedule: S(jb) and PV(ib) are interleaved (stagger 2) so PV matmuls keep
the PE busy while ACT drains exps; small constants ride in two packed
DMAs so the weight loads aren't stuck behind seven tiny descriptors; pm
biases for all batches load once. DMA queues are program-ordered per
engine: x prefetch on nc.sync, consts/weights on nc.scalar, stores on
nc.gpsimd. Dummy warmup matmuls pre-warm the PE clock-gate while batch-0
inputs stream in.
"""

import os
from contextlib import ExitStack

import numpy as np
import ml_dtypes

import concourse.tile as tile
from concourse import bacc, mybir
from concourse import bass_utils

F32 = mybir.dt.float32
F16 = mybir.dt.float16
F8 = mybir.dt.float8e4
E4 = ml_dtypes.float8_e4m3
AF = mybir.ActivationFunctionType
DRM = mybir.MatmulPerfMode.DoubleRow

N_CORES = 8
B = 32
L = 1024
C = 1024  # d_model
D = 512
P = 128
NB = B // N_CORES  # batches per core
CT = C // P  # 8 contraction slabs
DT = D // P  # 4 d slabs
LT = L // P  # 8 l/j/i tiles
SCALE = float(D) ** -0.5
NEG = -30000.0
WSCALE = 16.0  # fp8 weight pre-scale (escapes e4m3 subnormals)

# packed fp32 const layout (columns)
_BQ2, _BK2 = 0, DT
_BQB, _BKB, _BVB = 2 * DT, 2 * DT + D, 2 * DT + 2 * D
F32PACK = 2 * DT + 3 * D


def build_program(nb: int = NB):
    """Build the per-core Bass program for nb batches."""
    nc = bacc.Bacc("TRN2", target_bir_lowering=False, debug=False,
                   num_devices=N_CORES)

    x8b = nc.dram_tensor("x8b", [nb, P, CT, L], F8, kind="ExternalInput").ap()
    xtb = nc.dram_tensor("xtb", [nb, P, CT, P], F16, kind="ExternalInput").ap()
    wq8 = nc.dram_tensor("wq8", [P, CT, D], F8, kind="ExternalInput").ap()
    wk8 = nc.dram_tensor("wk8", [P, CT, D], F8, kind="ExternalInput").ap()
    wv8 = nc.dram_tensor("wv8", [P, CT, D], F8, kind="ExternalInput").ap()
    wqT = nc.dram_tensor("wqT", [C, D], F16, kind="ExternalInput").ap()
    wkT = nc.dram_tensor("wkT", [C, D], F16, kind="ExternalInput").ap()
    wvT = nc.dram_tensor("wvT", [C, D], F16, kind="ExternalInput").ap()
    cfs = nc.dram_tensor("cfs", [P, 2 * DT], F32, kind="ExternalInput").ap()
    cf32 = nc.dram_tensor("cf32", [P, 3 * D], F32, kind="ExternalInput").ap()
    cf16 = nc.dram_tensor("cf16", [P, 2 * P], F16, kind="ExternalInput").ap()
    pmt = nc.dram_tensor("pmt", [P, nb, LT], F32, kind="ExternalInput").ap()
    out = nc.dram_tensor("out", [nb, L, D], F16, kind="ExternalOutput").ap()

    with tile.TileContext(nc) as tc, ExitStack() as ctx:
        const = ctx.enter_context(tc.tile_pool(name="const", bufs=1))
        x8_pool = ctx.enter_context(tc.tile_pool(name="x8", bufs=2))
        xt_pool = ctx.enter_context(tc.tile_pool(name="xt", bufs=2))
        qk_pool = ctx.enter_context(tc.tile_pool(name="qk", bufs=2))
        blk_pool = ctx.enter_context(tc.tile_pool(name="blk", bufs=2))
        v_pool = ctx.enter_context(tc.tile_pool(name="v", bufs=2))
        pt_pool = ctx.enter_context(tc.tile_pool(name="pt", bufs=2))
        o_pool = ctx.enter_context(tc.tile_pool(name="o", bufs=3))
        sm_pool = ctx.enter_context(tc.tile_pool(name="sm", bufs=4))
        mm_ps = ctx.enter_context(tc.tile_pool(name="mmps", bufs=4, space="PSUM"))
        o_ps = ctx.enter_context(tc.tile_pool(name="ops", bufs=2, space="PSUM"))
        r_ps = ctx.enter_context(tc.tile_pool(name="rps", bufs=2, space="PSUM"))

        # --- constants on the scalar HWDGE queue (sync is for x prefetch).
        # wq8 first (it gates the very first matmul), tiny packs next. ---
        wq_sb = const.tile([P, CT, D], F8)
        nc.scalar.dma_start(wq_sb[:], wq8[:])
        cfs_sb = const.tile([P, 2 * DT], F32)
        nc.scalar.dma_start(cfs_sb[:], cfs[:])
        wk_sb = const.tile([P, CT, D], F8)
        nc.scalar.dma_start(wk_sb[:], wk8[:])
        wv8_sb = const.tile([P, CT, D], F8)
        nc.scalar.dma_start(wv8_sb[:], wv8[:])
        pm_all = const.tile([P, nb, LT], F32)
        nc.scalar.dma_start(pm_all[:], pmt[:])
        cf32_sb = const.tile([P, 3 * D], F32)
        nc.scalar.dma_start(cf32_sb[:], cf32[:])
        cf16_sb = const.tile([P, 2 * P], F16)
        nc.scalar.dma_start(cf16_sb[:], cf16[:])
        wq16_sb = const.tile([P, CT, D], F16)
        nc.scalar.dma_start(wq16_sb[:],
                            wqT.rearrange("(t p) d -> p t d", p=P))
        wk16_sb = const.tile([P, CT, D], F16)
        nc.scalar.dma_start(wk16_sb[:],
                            wkT.rearrange("(t p) d -> p t d", p=P))
        wv16_sb = const.tile([P, CT, D], F16)
        nc.scalar.dma_start(wv16_sb[:],
                            wvT.rearrange("(t p) d -> p t d", p=P))

        bq_sb = cfs_sb[:, 0:DT]
        bk_sb = cfs_sb[:, DT:2 * DT]
        bqb_sb = cf32_sb[:, 0:D]
        bkb_sb = cf32_sb[:, D:2 * D]
        bvb_sb = cf32_sb[:, 2 * D:3 * D]  # 16*bv broadcast
        tri_sb = cf16_sb[:, 0:P]
        idm_sb = cf16_sb[:, P:2 * P]

        ones16_sb = const.tile([P, 1], F16)
        nc.vector.memset(ones16_sb[:], WSCALE)
        ones8_sb = const.tile([P, 2, 16], F8)
        nc.vector.memset(ones8_sb[:], WSCALE)

        # PE warmup: dummy matmuls with no input deps keep the PE busy while
        # wq8/x8 stream in, so the HAM clock-gate is at 2.4 GHz when the
        # real stream starts.
        warm_sb = const.tile([P, 512], F16)
        nc.vector.memset(warm_sb[:], 0.0)
        for w in range(9):
            wps = mm_ps.tile([P, 512], F32, tag="ps", name=f"warm{w}")
            nc.tensor.matmul(wps[:], warm_sb[:, 0:P], warm_sb[:],
                             start=True, stop=True)

        for b in range(nb):
            # --- X fp8 slab-major [128, CT, L] for Q/K/V ---
            x8 = x8_pool.tile([P, CT, L], F8, tag="x8", name=f"x8_{b}")
            if b == 0:
                # first batch: l<512 halves first so Q lc=0 starts earlier
                nc.sync.dma_start(x8[:, :, 0:512], x8b[b][:, :, 0:512])
                nc.sync.dma_start(x8[:, :, 512:L], x8b[b][:, :, 512:L])
            else:
                nc.sync.dma_start(x8[:], x8b[b])
            # --- X^T fp16 l<128 slab-major tile for the early block ---
            xt16 = xt_pool.tile([P, CT, P], F16, tag="xt", name=f"xt_{b}")
            nc.sync.dma_start(xt16[:], xtb[b])

            # --- Q^T, K^T: fp8 [128, DT, L] tiles; DoubleRow over 2-slab
            # pairs; descale+bias folded into the DVE drain ---
            qt = qk_pool.tile([P, DT, L], F8, tag="qt", name=f"qt_{b}")
            kt = qk_pool.tile([P, DT, L], F8, tag="kt", name=f"kt_{b}")
            for name, w_sb, b_sb, dst in (("q", wq_sb, bq_sb, qt),
                                          ("k", wk_sb, bk_sb, kt)):
                if b == 0 and name == "q":
                    # lc-outer so all lc=0 groups run on the early halves
                    order = [(dt, lc) for lc in range(L // 512)
                             for dt in range(DT)]
                else:
                    order = [(dt, lc) for dt in range(DT)
                             for lc in range(L // 512)]
                for dt, lc in order:
                    ps = mm_ps.tile([P, 512], F32, tag="ps",
                                    name=f"{name}ps{dt}_{lc}_{b}")
                    for s in range(CT // 2):
                        nc.tensor.matmul(
                            ps[:],
                            w_sb[:, 2 * s:2 * s + 2, dt * P:(dt + 1) * P],
                            x8[:, 2 * s:2 * s + 2, lc * 512:(lc + 1) * 512],
                            start=(s == 0), stop=(s == CT // 2 - 1),
                            perf_mode=DRM)
                    nc.vector.tensor_scalar(
                        dst[:, dt, lc * 512:(lc + 1) * 512], ps[:],
                        1.0 / WSCALE, b_sb[:, dt:dt + 1],
                        mybir.AluOpType.mult, mybir.AluOpType.add)

            # --- phases: V fp8 pair tiles [128l, 2, 512d] (DoubleRow, 16x
            # value domain); early-block q,k [l<128, d] fp16 (x-stationary,
            # N=512 — rows i<128 need an accurate fp16 path, fp8 logit noise
            # doesn't average out in few-term softmaxes) + 8 PE transposes
            # to d-major; v0 as fp16 for the ib=0 rim. For b=0, V8 runs
            # first (wv8 lands ~7us; the fat fp16 weights land ~19us). ---
            blkq = blk_pool.tile([P, D], F16, tag="blkq", name=f"blkq_{b}")
            blkk = blk_pool.tile([P, D], F16, tag="blkk", name=f"blkk_{b}")
            qt16 = blk_pool.tile([P, DT, P], F16, tag="qt16", name=f"qt16_{b}")
            kt16 = blk_pool.tile([P, DT, P], F16, tag="kt16", name=f"kt16_{b}")
            v8p = [v_pool.tile([P, 2, D], F8, tag=f"v8p{t}", name=f"v8p{t}_{b}")
                   for t in range(LT // 2)]
            v016 = v_pool.tile([P, D], F16, tag="v016", name=f"v016_{b}")
            tponder = [(blkq, qt16, ds) for ds in range(DT)] + \
                      [(blkk, kt16, ds) for ds in range(DT)]

            def emit_blk():
                for w16_sb, bb_sb, blk in ((wq16_sb, bqb_sb, blkq),
                                           (wk16_sb, bkb_sb, blkk)):
                    ps = mm_ps.tile([P, D], F32, tag="ps", name=f"blkps_{b}")
                    for ct in range(CT):
                        nc.tensor.matmul(ps[:], xt16[:, ct, :],
                                         w16_sb[:, ct, :],
                                         start=(ct == 0), stop=(ct == CT - 1))
                    nc.vector.tensor_add(blk[:], ps[:], bb_sb)

            def emit_v8(lt, transp):
                ps = mm_ps.tile([P, D], F32, tag="ps", name=f"vps{lt}_{b}")
                for s in range(CT // 2):
                    nc.tensor.matmul(
                        ps[:],
                        x8[:, 2 * s:2 * s + 2, lt * P:(lt + 1) * P],
                        wv8_sb[:, 2 * s:2 * s + 2, :],
                        start=(s == 0), stop=(s == CT // 2 - 1),
                        perf_mode=DRM)
                nc.vector.tensor_add(v8p[lt // 2][:, lt % 2, :], ps[:],
                                     bvb_sb)
                if transp and lt < len(tponder):
                    emit_transpose(lt)

            def emit_transpose(k):
                blk, dstT, ds = tponder[k]
                tps = mm_ps.tile([P, P], F16, tag="ps", name=f"tps{k}_{b}")
                nc.tensor.transpose(tps[:], blk[:, ds * P:(ds + 1) * P],
                                    idm_sb)
                nc.vector.tensor_copy(dstT[:, ds, :], tps[:])

            def emit_v016():
                ps = mm_ps.tile([P, D], F32, tag="ps", name=f"v016ps_{b}")
                for ct in range(CT):
                    nc.tensor.matmul(ps[:], xt16[:, ct, :], wv16_sb[:, ct, :],
                                     start=(ct == 0), stop=(ct == CT - 1))
                nc.vector.tensor_add(v016[:], ps[:], bvb_sb)


            # --- S^T tiles + exp -> P^T fp8 pair tiles (causal: only
            # i >= j0 computed), interleaved with PV (stagger 2) so PV
            # matmuls keep the PE busy while ACT drains exps ---
            ptp = [pt_pool.tile([P, 2, L], F8, tag=f"ptp{t}",
                                name=f"ptp{t}_{b}")
                   for t in range(LT // 2)]
            pt0h = pt_pool.tile([P, P], F16, tag="pt0h", name=f"pt0h_{b}")

            def emit_s(jb):
                j0 = jb * P
                tIdx, slab = jb // 2, jb % 2
                i0 = j0
                if jb == 0:
                    # rows i<128: accurate fp16 path from qt16/kt16
                    ps = mm_ps.tile([P, P], F32, tag="ps", name=f"sps0h_{b}")
                    for dt in range(DT):
                        nc.tensor.matmul(ps[:], kt16[:, dt, :], qt16[:, dt, :],
                                         start=(dt == 0), stop=(dt == DT - 1))
                    nc.scalar.activation(pt0h[:], ps[:], AF.Exp,
                                         bias=pm_all[:, b, 0:1], scale=SCALE)
                    nc.vector.tensor_mul(pt0h[:], pt0h[:], tri_sb)
                    i0 = P
                while i0 < L:
                    n = min((i0 // 512 + 1) * 512, L) - i0
                    ps = mm_ps.tile([P, n], F32, tag="ps",
                                    name=f"sps{jb}_{i0}_{b}")
                    for s in range(DT // 2):
                        nc.tensor.matmul(
                            ps[:],
                            kt[:, 2 * s:2 * s + 2, j0:j0 + P],
                            qt[:, 2 * s:2 * s + 2, i0:i0 + n],
                            start=(s == 0), stop=(s == DT // 2 - 1),
                            perf_mode=DRM)
                    nc.scalar.activation(ptp[tIdx][:, slab, i0:i0 + n], ps[:],
                                         AF.Exp, bias=pm_all[:, b, jb:jb + 1],
                                         scale=SCALE)
                    i0 += n
                if jb > 0:
                    # mask the diagonal tile: keep j<=i
                    nc.vector.tensor_mul(ptp[tIdx][:, slab, j0:j0 + P],
                                         ptp[tIdx][:, slab, j0:j0 + P],
                                         tri_sb)

            def emit_pv(ib):
                i0 = ib * P
                ops = o_ps.tile([P, D], F32, tag="op", name=f"ops{ib}_{b}")
                rps = r_ps.tile([P, 1], F32, tag="rp", name=f"rps{ib}_{b}")
                if ib == 0:
                    nc.tensor.matmul(ops[:], pt0h[:], v016[:],
                                     start=True, stop=True)
                    nc.tensor.matmul(rps[:], pt0h[:], ones16_sb[:],
                                     start=True, stop=True)
                else:
                    # fp8 DoubleRow over jb pairs; odd tail as plain fp8.
                    # rowsum piggybacks with 16.0-ones: out = (sum p*16v)
                    # / (16*sum p) — the 16x V domain cancels exactly.
                    npair = (ib + 1) // 2
                    leftover = (ib + 1) % 2
                    last = npair + leftover - 1
                    for t in range(npair):
                        st, sp = (t == 0), (t == last and not leftover)
                        pT = ptp[t][:, :, i0:i0 + P]
                        nc.tensor.matmul(ops[:], pT, v8p[t][:],
                                         start=st, stop=sp, perf_mode=DRM)
                        nc.tensor.matmul(rps[:], pT, ones8_sb[:, :, 0:1],
                                         start=st, stop=sp, perf_mode=DRM)
                    if leftover:
                        t = npair
                        pT = ptp[t][:, 0, i0:i0 + P]
                        nc.tensor.matmul(ops[:], pT, v8p[t][:, 0, :],
                                         start=(npair == 0), stop=True)
                        nc.tensor.matmul(rps[:], pT, ones8_sb[:, 0, 0:1],
                                         start=(npair == 0), stop=True)
                rec = sm_pool.tile([P, 1], F32, tag="rec", name=f"rec{ib}_{b}")
                nc.vector.reciprocal(rec[:], rps[:])
                o_sb = o_pool.tile([P, D], F16, tag="ot", name=f"o{ib}_{b}")
                # relu(O'/rowsum) on DVE: (in0 * rec) max 0 — keeps ACT free
                # for the exps, whose latency gates S-phase PSUM slot reuse
                nc.vector.tensor_scalar(o_sb[:], ops[:], rec[:], 0.0,
                                        mybir.AluOpType.mult,
                                        mybir.AluOpType.max)
                # SWDGE so stores never head-of-line-block the x prefetch;
                # last batch has no prefetch left, so use the faster HWDGE
                if b == nb - 1:
                    eng = nc.sync if ib % 2 == 0 else nc.scalar
                    eng.dma_start(out[b, i0:i0 + P, :], o_sb[:])
                else:
                    nc.gpsimd.dma_start(out[b, i0:i0 + P, :], o_sb[:])

            emit_blk()
            for lt in range(LT):
                emit_v8(lt, transp=True)
            emit_v016()
            STAG = 2
            for jb in range(LT):
                emit_s(jb)
                if jb >= STAG:
                    emit_pv(jb - STAG)
            for ib in range(LT - STAG, LT):
                emit_pv(ib)

    nc.compile()
    return nc


def _prep_host(x, Wq, bq, Wk, bk, Wv, bv, mask):
    bf = np.float16
    f32 = np.float32
    xT = np.ascontiguousarray(x.transpose(0, 2, 1))  # [B, C, L]
    xs = xT.reshape(B, CT, P, L)
    # x fp8 slab-major [B, 128, CT, L] for the DoubleRow paths
    x8b = np.ascontiguousarray(xs.transpose(0, 2, 1, 3)).astype(E4)
    # x fp16 l<128 slab-major [B, 128, CT, 128] for the early block
    xtb = np.ascontiguousarray(
        xs[:, :, :, 0:P].transpose(0, 2, 1, 3)).astype(bf)

    def pack8(W):
        return np.ascontiguousarray(
            (W.T * WSCALE).reshape(CT, P, D).transpose(1, 0, 2)).astype(E4)

    wq8, wk8, wv8 = pack8(Wq), pack8(Wk), pack8(Wv)
    wqT = np.ascontiguousarray(Wq.T).astype(bf)
    wkT = np.ascontiguousarray(Wk.T).astype(bf)
    wvT = np.ascontiguousarray(Wv.T * WSCALE).astype(bf)  # 16x domain

    cfs = np.zeros((P, 2 * DT), dtype=f32)
    cfs[:, 0:DT] = bq.astype(f32).reshape(DT, P).T
    cfs[:, DT:2 * DT] = bk.astype(f32).reshape(DT, P).T
    cf32 = np.zeros((P, 3 * D), dtype=f32)
    cf32[:, 0:D] = bq.astype(f32)[None, :]
    cf32[:, D:2 * D] = bk.astype(f32)[None, :]
    cf32[:, 2 * D:3 * D] = bv.astype(f32)[None, :] * WSCALE
    cf16 = np.zeros((P, 2 * P), dtype=bf)
    cf16[:, 0:P] = (np.arange(P)[:, None] <= np.arange(P)[None, :])
    cf16[:, P:2 * P] = np.eye(P)

    pm = np.where(mask[:, 0, :] != 0, 0.0, NEG).astype(f32)  # [B, L]
    pmt = np.ascontiguousarray(
        pm.reshape(B, LT, P).transpose(2, 0, 1))  # [P, B, LT]
    return x8b, xtb, wq8, wk8, wv8, wqT, wkT, wvT, cfs, cf32, cf16, pmt


_NC_CACHE = {}


def kernel(x, Wq, bq, Wk, bk, Wv, bv, mask):
    x = np.asarray(x)
    Wq, bq = np.asarray(Wq), np.asarray(bq)
    Wk, bk = np.asarray(Wk), np.asarray(bk)
    Wv, bv = np.asarray(Wv), np.asarray(bv)
    mask = np.asarray(mask)

    (x8b, xtb, wq8, wk8, wv8, wqT, wkT, wvT, cfs, cf32, cf16, pmt) = _prep_host(
        x, Wq, bq, Wk, bk, Wv, bv, mask)

    if "nc" not in _NC_CACHE:
        _NC_CACHE["nc"] = build_program(NB)
    nc = _NC_CACHE["nc"]

    in_maps = []
    for c in range(N_CORES):
        s = slice(c * NB, (c + 1) * NB)
        in_maps.append({
            "x8b": np.ascontiguousarray(x8b[s]),
            "xtb": np.ascontiguousarray(xtb[s]),
            "wq8": wq8, "wk8": wk8, "wv8": wv8,
            "wqT": wqT, "wkT": wkT, "wvT": wvT,
            "cfs": cfs, "cf32": cf32, "cf16": cf16,
            "pmt": np.ascontiguousarray(pmt[:, s]),
        })

    res = bass_utils.run_bass_kernel_spmd(
        nc, in_maps, core_ids=list(range(N_CORES)),
        trace=bool(int(os.environ.get("KERNEL_TRACE", "0"))),
    )
    if os.environ.get("KERNEL_RESULT_HOOK"):
        _NC_CACHE["last_result"] = res

    return np.concatenate([res.results[c]["out"] for c in range(N_CORES)],
                          axis=0).astype(np.float32)


# revision 23
# speedup vs baseline: 1.1745x; 1.1745x over previous
"""Causal attention (QKV proj + softmax + PV + ReLU) on 8 trn2 NeuronCores.

Sharding: data-parallel over batch B=32 -> 4 batches per core; projection
weights replicated.

Dtypes: everything that can tolerate it runs in fp8-e4m3 with DoubleRow
matmuls (two 128-row contraction slabs per pass -> 2x PE throughput vs
fp16): the Q/K projections, S=K^T.T@Q^T, the V projection, and P@V.
Accumulation is always fp32 in PSUM. Error control:
  - weights are pre-scaled by 16 on the host so their U(-1/32,1/32)
    entries escape e4m3's subnormal range (descale folded into drains; for
    V the 16x rides the value domain and cancels exactly in the rowsum
    normalization, via 16.0-valued `ones`).
  - rows i<128 get a full fp16 path (fp16 projections of q,k,v for l<128,
    fp16 S block, fp16 PV for ib=0): few-term softmax rows pass quant
    noise straight to the output, everything later averages it away.
    Measured rel-err 6.7e-3 vs the 2e-2 gate.
  - the fp16 early-block q,k come from V-style x-stationary N=512 matmuls
    (full PE rate) + 8 PE transposes to d-major, interleaved into the V
    phase so transpose LDWEIGHTS hide under N=512 streams.

Schedule: S(jb) and PV(ib) are interleaved (stagger 2) so PV matmuls keep
the PE busy while ACT drains exps; small constants ride in two packed
DMAs so the weight loads aren't stuck behind seven tiny descriptors; pm
biases for all batches load once. DMA queues are program-ordered per
engine: x prefetch on nc.sync, consts/weights on nc.scalar, stores on
nc.gpsimd. Dummy warmup matmuls pre-warm the PE clock-gate while batch-0
inputs stream in.
"""

import os
from contextlib import ExitStack

import numpy as np
import ml_dtypes

import concourse.tile as tile
from concourse import bacc, mybir
from concourse import bass_utils

F32 = mybir.dt.float32
F16 = mybir.dt.float16
F8 = mybir.dt.float8e4
E4 = ml_dtypes.float8_e4m3
AF = mybir.ActivationFunctionType
DRM = mybir.MatmulPerfMode.DoubleRow

N_CORES = 8
B = 32
L = 1024
C = 1024  # d_model
D = 512
P = 128
NB = B // N_CORES  # batches per core
CT = C // P  # 8 contraction slabs
DT = D // P  # 4 d slabs
LT = L // P  # 8 l/j/i tiles
SCALE = float(D) ** -0.5
NEG = -30000.0
WSCALE = 16.0  # fp8 weight pre-scale (escapes e4m3 subnormals)

# packed fp32 const layout (columns)
_BQ2, _BK2 = 0, DT
_BQB, _BKB, _BVB = 2 * DT, 2 * DT + D, 2 * DT + 2 * D
F32PACK = 2 * DT + 3 * D


def build_program(nb: int = NB):
    """Build the per-core Bass program for nb batches."""
    nc = bacc.Bacc("TRN2", target_bir_lowering=False, debug=False,
                   num_devices=N_CORES)

    x8b = nc.dram_tensor("x8b", [nb, P, CT, L], F8, kind="ExternalInput").ap()
    xtb = nc.dram_tensor("xtb", [nb, P, CT, P], F16, kind="ExternalInput").ap()
    wq8 = nc.dram_tensor("wq8", [P, CT, D], F8, kind="ExternalInput").ap()
    wk8 = nc.dram_tensor("wk8", [P, CT, D], F8, kind="ExternalInput").ap()
    wv8 = nc.dram_tensor("wv8", [P, CT, D], F8, kind="ExternalInput").ap()
    wqT = nc.dram_tensor("wqT", [C, D], F16, kind="ExternalInput").ap()
    wkT = nc.dram_tensor("wkT", [C, D], F16, kind="ExternalInput").ap()
    wvT = nc.dram_tensor("wvT", [C, D], F16, kind="ExternalInput").ap()
    cfs = nc.dram_tensor("cfs", [P, 2 * DT], F32, kind="ExternalInput").ap()
    cf32 = nc.dram_tensor("cf32", [P, 3 * D], F32, kind="ExternalInput").ap()
    cf16 = nc.dram_tensor("cf16", [P, 2 * P], F16, kind="ExternalInput").ap()
    pmt = nc.dram_tensor("pmt", [P, nb, LT], F32, kind="ExternalInput").ap()
    out = nc.dram_tensor("out", [nb, L, D], F16, kind="ExternalOutput").ap()

    with tile.TileContext(nc) as tc, ExitStack() as ctx:
        const = ctx.enter_context(tc.tile_pool(name="const", bufs=1))
        x8_pool = ctx.enter_context(tc.tile_pool(name="x8", bufs=2))
        xt_pool = ctx.enter_context(tc.tile_pool(name="xt", bufs=2))
        qk_pool = ctx.enter_context(tc.tile_pool(name="qk", bufs=2))
        blk_pool = ctx.enter_context(tc.tile_pool(name="blk", bufs=2))
        v_pool = ctx.enter_context(tc.tile_pool(name="v", bufs=2))
        pt_pool = ctx.enter_context(tc.tile_pool(name="pt", bufs=2))
        o_pool = ctx.enter_context(tc.tile_pool(name="o", bufs=3))
        sm_pool = ctx.enter_context(tc.tile_pool(name="sm", bufs=4))
        mm_ps = ctx.enter_context(tc.tile_pool(name="mmps", bufs=4, space="PSUM"))
        o_ps = ctx.enter_context(tc.tile_pool(name="ops", bufs=2, space="PSUM"))
        r_ps = ctx.enter_context(tc.tile_pool(name="rps", bufs=2, space="PSUM"))

        # --- constants on the scalar HWDGE queue (sync is for x prefetch).
        # wq8 first (it gates the very first matmul), tiny packs next. ---
        wq_sb = const.tile([P, CT, D], F8)
        nc.scalar.dma_start(wq_sb[:], wq8[:])
        cfs_sb = const.tile([P, 2 * DT], F32)
        nc.scalar.dma_start(cfs_sb[:], cfs[:])
        wk_sb = const.tile([P, CT, D], F8)
        nc.scalar.dma_start(wk_sb[:], wk8[:])
        wv8_sb = const.tile([P, CT, D], F8)
        nc.scalar.dma_start(wv8_sb[:], wv8[:])
        pm_all = const.tile([P, nb, LT], F32)
        nc.scalar.dma_start(pm_all[:], pmt[:])
        cf32_sb = const.tile([P, 3 * D], F32)
        nc.scalar.dma_start(cf32_sb[:], cf32[:])
        cf16_sb = const.tile([P, 2 * P], F16)
        nc.scalar.dma_start(cf16_sb[:], cf16[:])
        wq16_sb = const.tile([P, CT, D], F16)
        nc.scalar.dma_start(wq16_sb[:],
                            wqT.rearrange("(t p) d -> p t d", p=P))
        wk16_sb = const.tile([P, CT, D], F16)
        nc.scalar.dma_start(wk16_sb[:],
                            wkT.rearrange("(t p) d -> p t d", p=P))
        wv16_sb = const.tile([P, CT, D], F16)
        nc.scalar.dma_start(wv16_sb[:],
                            wvT.rearrange("(t p) d -> p t d", p=P))

        bq_sb = cfs_sb[:, 0:DT]
        bk_sb = cfs_sb[:, DT:2 * DT]
        bqb_sb = cf32_sb[:, 0:D]
        bkb_sb = cf32_sb[:, D:2 * D]
        bvb_sb = cf32_sb[:, 2 * D:3 * D]  # 16*bv broadcast
        tri_sb = cf16_sb[:, 0:P]
        idm_sb = cf16_sb[:, P:2 * P]

        ones16_sb = const.tile([P, 1], F16)
        nc.vector.memset(ones16_sb[:], WSCALE)
        ones8_sb = const.tile([P, 2, 16], F8)
        nc.vector.memset(ones8_sb[:], WSCALE)

        # PE warmup: dummy matmuls with no input deps keep the PE busy while
        # wq8/x8 stream in, so the HAM clock-gate is at 2.4 GHz when the
        # real stream starts.
        warm_sb = const.tile([P, 512], F16)
        nc.vector.memset(warm_sb[:], 0.0)
        for w in range(9):
            wps = mm_ps.tile([P, 512], F32, tag="ps", name=f"warm{w}")
            nc.tensor.matmul(wps[:], warm_sb[:, 0:P], warm_sb[:],
                             start=True, stop=True)

        for b in range(nb):
            # --- X fp8 slab-major [128, CT, L] for Q/K/V ---
            x8 = x8_pool.tile([P, CT, L], F8, tag="x8", name=f"x8_{b}")
            if b == 0:
                # first batch: l<512 halves first so Q lc=0 starts earlier
                nc.sync.dma_start(x8[:, :, 0:512], x8b[b][:, :, 0:512])
                nc.sync.dma_start(x8[:, :, 512:L], x8b[b][:, :, 512:L])
            else:
                nc.sync.dma_start(x8[:], x8b[b])
            # --- X^T fp16 l<128 slab-major tile for the early block ---
            xt16 = xt_pool.tile([P, CT, P], F16, tag="xt", name=f"xt_{b}")
            nc.sync.dma_start(xt16[:], xtb[b])

            # --- Q^T, K^T: fp8 [128, DT, L] tiles; DoubleRow over 2-slab
            # pairs; descale+bias folded into the DVE drain ---
            qt = qk_pool.tile([P, DT, L], F8, tag="qt", name=f"qt_{b}")
            kt = qk_pool.tile([P, DT, L], F8, tag="kt", name=f"kt_{b}")
            for name, w_sb, b_sb, dst in (("q", wq_sb, bq_sb, qt),
                                          ("k", wk_sb, bk_sb, kt)):
                if b == 0 and name == "q":
                    # lc-outer so all lc=0 groups run on the early halves
                    order = [(dt, lc) for lc in range(L // 512)
                             for dt in range(DT)]
                else:
                    order = [(dt, lc) for dt in range(DT)
                             for lc in range(L // 512)]
                for dt, lc in order:
                    ps = mm_ps.tile([P, 512], F32, tag="ps",
                                    name=f"{name}ps{dt}_{lc}_{b}")
                    for s in range(CT // 2):
                        nc.tensor.matmul(
                            ps[:],
                            w_sb[:, 2 * s:2 * s + 2, dt * P:(dt + 1) * P],
                            x8[:, 2 * s:2 * s + 2, lc * 512:(lc + 1) * 512],
                            start=(s == 0), stop=(s == CT // 2 - 1),
                            perf_mode=DRM)
                    nc.vector.tensor_scalar(
                        dst[:, dt, lc * 512:(lc + 1) * 512], ps[:],
                        1.0 / WSCALE, b_sb[:, dt:dt + 1],
                        mybir.AluOpType.mult, mybir.AluOpType.add)

            # --- phases: V fp8 pair tiles [128l, 2, 512d] (DoubleRow, 16x
            # value domain); early-block q,k [l<128, d] fp16 (x-stationary,
            # N=512 — rows i<128 need an accurate fp16 path, fp8 logit noise
            # doesn't average out in few-term softmaxes) + 8 PE transposes
            # to d-major; v0 as fp16 for the ib=0 rim. For b=0, V8 runs
            # first (wv8 lands ~7us; the fat fp16 weights land ~19us). ---
            blkq = blk_pool.tile([P, D], F16, tag="blkq", name=f"blkq_{b}")
            blkk = blk_pool.tile([P, D], F16, tag="blkk", name=f"blkk_{b}")
            qt16 = blk_pool.tile([P, DT, P], F16, tag="qt16", name=f"qt16_{b}")
            kt16 = blk_pool.tile([P, DT, P], F16, tag="kt16", name=f"kt16_{b}")
            v8p = [v_pool.tile([P, 2, D], F8, tag=f"v8p{t}", name=f"v8p{t}_{b}")
                   for t in range(LT // 2)]
            v016 = v_pool.tile([P, D], F16, tag="v016", name=f"v016_{b}")
            tponder = [(blkq, qt16, ds) for ds in range(DT)] + \
                      [(blkk, kt16, ds) for ds in range(DT)]

            def emit_blk():
                for w16_sb, bb_sb, blk in ((wq16_sb, bqb_sb, blkq),
                                           (wk16_sb, bkb_sb, blkk)):
                    ps = mm_ps.tile([P, D], F32, tag="ps", name=f"blkps_{b}")
                    for ct in range(CT):
                        nc.tensor.matmul(ps[:], xt16[:, ct, :],
                                         w16_sb[:, ct, :],
                                         start=(ct == 0), stop=(ct == CT - 1))
                    nc.vector.tensor_add(blk[:], ps[:], bb_sb)

            def emit_v8(lt, transp):
                ps = mm_ps.tile([P, D], F32, tag="ps", name=f"vps{lt}_{b}")
                for s in range(CT // 2):
                    nc.tensor.matmul(
                        ps[:],
                        x8[:, 2 * s:2 * s + 2, lt * P:(lt + 1) * P],
                        wv8_sb[:, 2 * s:2 * s + 2, :],
                        start=(s == 0), stop=(s == CT // 2 - 1),
                        perf_mode=DRM)
                nc.vector.tensor_add(v8p[lt // 2][:, lt % 2, :], ps[:],
                                     bvb_sb)
                if transp and lt < len(tponder):
                    emit_transpose(lt)

            def emit_transpose(k):
                blk, dstT, ds = tponder[k]
                tps = mm_ps.tile([P, P], F16, tag="ps", name=f"tps{k}_{b}")
                nc.tensor.transpose(tps[:], blk[:, ds * P:(ds + 1) * P],
                                    idm_sb)
                nc.vector.tensor_copy(dstT[:, ds, :], tps[:])

            def emit_v016():
                ps = mm_ps.tile([P, D], F32, tag="ps", name=f"v016ps_{b}")
                for ct in range(CT):
                    nc.tensor.matmul(ps[:], xt16[:, ct, :], wv16_sb[:, ct, :],
                                     start=(ct == 0), stop=(ct == CT - 1))
                nc.vector.tensor_add(v016[:], ps[:], bvb_sb)


            # --- S^T tiles + exp -> P^T fp8 pair tiles (causal: only
            # i >= j0 computed), interleaved with PV (stagger 2) so PV
            # matmuls keep the PE busy while ACT drains exps ---
            ptp = [pt_pool.tile([P, 2, L], F8, tag=f"ptp{t}",
                                name=f"ptp{t}_{b}")
                   for t in range(LT // 2)]
            pt0h = pt_pool.tile([P, P], F16, tag="pt0h", name=f"pt0h_{b}")

            def emit_s(jb):
                j0 = jb * P
                tIdx, slab = jb // 2, jb % 2
                i0 = j0
                if jb == 0:
                    # rows i<128: accurate fp16 path from qt16/kt16
                    ps = mm_ps.tile([P, P], F32, tag="ps", name=f"sps0h_{b}")
                    for dt in range(DT):
                        nc.tensor.matmul(ps[:], kt16[:, dt, :], qt16[:, dt, :],
                                         start=(dt == 0), stop=(dt == DT - 1))
                    nc.scalar.activation(pt0h[:], ps[:], AF.Exp,
                                         bias=pm_all[:, b, 0:1], scale=SCALE)
                    nc.vector.tensor_mul(pt0h[:], pt0h[:], tri_sb)
                    i0 = P
                while i0 < L:
                    n = min((i0 // 512 + 1) * 512, L) - i0
                    ps = mm_ps.tile([P, n], F32, tag="ps",
                                    name=f"sps{jb}_{i0}_{b}")
                    for s in range(DT // 2):
                        nc.tensor.matmul(
                            ps[:],
                            kt[:, 2 * s:2 * s + 2, j0:j0 + P],
                            qt[:, 2 * s:2 * s + 2, i0:i0 + n],
                            start=(s == 0), stop=(s == DT // 2 - 1),
                            perf_mode=DRM)
                    nc.scalar.activation(ptp[tIdx][:, slab, i0:i0 + n], ps[:],
                                         AF.Exp, bias=pm_all[:, b, jb:jb + 1],
                                         scale=SCALE)
                    i0 += n
                if jb > 0:
                    # mask the diagonal tile: keep j<=i
                    nc.vector.tensor_mul(ptp[tIdx][:, slab, j0:j0 + P],
                                         ptp[tIdx][:, slab, j0:j0 + P],
                                         tri_sb)

            def emit_pv(ib):
                i0 = ib * P
                ops = o_ps.tile([P, D], F32, tag="op", name=f"ops{ib}_{b}")
                rps = r_ps.tile([P, 1], F32, tag="rp", name=f"rps{ib}_{b}")
                if ib == 0:
                    nc.tensor.matmul(ops[:], pt0h[:], v016[:],
                                     start=True, stop=True)
                    nc.tensor.matmul(rps[:], pt0h[:], ones16_sb[:],
                                     start=True, stop=True)
                else:
                    # fp8 DoubleRow over jb pairs; odd tail as plain fp8.
                    # rowsum piggybacks with 16.0-ones: out = (sum p*16v)
                    # / (16*sum p) — the 16x V domain cancels exactly.
                    npair = (ib + 1) // 2
                    leftover = (ib + 1) % 2
                    last = npair + leftover - 1
                    for t in range(npair):
                        st, sp = (t == 0), (t == last and not leftover)
                        pT = ptp[t][:, :, i0:i0 + P]
                        nc.tensor.matmul(ops[:], pT, v8p[t][:],
                                         start=st, stop=sp, perf_mode=DRM)
                        nc.tensor.matmul(rps[:], pT, ones8_sb[:, :, 0:1],
                                         start=st, stop=sp, perf_mode=DRM)
                    if leftover:
                        t = npair
                        pT = ptp[t][:, 0, i0:i0 + P]
                        nc.tensor.matmul(ops[:], pT, v8p[t][:, 0, :],
                                         start=(npair == 0), stop=True)
                        nc.tensor.matmul(rps[:], pT, ones8_sb[:, 0, 0:1],
                                         start=(npair == 0), stop=True)
                rec = sm_pool.tile([P, 1], F32, tag="rec", name=f"rec{ib}_{b}")
                nc.vector.reciprocal(rec[:], rps[:])
                o_sb = o_pool.tile([P, D], F16, tag="ot", name=f"o{ib}_{b}")
                # relu(O'/rowsum) on DVE: (in0 * rec) max 0 — keeps ACT free
                # for the exps, whose latency gates S-phase PSUM slot reuse
                nc.vector.tensor_scalar(o_sb[:], ops[:], rec[:], 0.0,
                                        mybir.AluOpType.mult,
                                        mybir.AluOpType.max)
                # SWDGE so stores never head-of-line-block the x prefetch;
                # last batch has no prefetch left, so use the faster HWDGE
                if b == nb - 1:
                    nc.sync.dma_start(out[b, i0:i0 + P, :], o_sb[:])
                else:
                    nc.gpsimd.dma_start(out[b, i0:i0 + P, :], o_sb[:])

            emit_blk()
            for lt in range(LT):
                emit_v8(lt, transp=True)
            emit_v016()
            STAG = 2
            for jb in range(LT):
                emit_s(jb)
                if jb >= STAG:
                    emit_pv(jb - STAG)
            for ib in range(LT - STAG, LT):
                emit_pv(ib)

    nc.compile()
    return nc


def _prep_host(x, Wq, bq, Wk, bk, Wv, bv, mask):
    bf = np.float16
    f32 = np.float32
    xT = np.ascontiguousarray(x.transpose(0, 2, 1))  # [B, C, L]
    xs = xT.reshape(B, CT, P, L)
    # x fp8 slab-major [B, 128, CT, L] for the DoubleRow paths
    x8b = np.ascontiguousarray(xs.transpose(0, 2, 1, 3)).astype(E4)
    # x fp16 l<128 slab-major [B, 128, CT, 128] for the early block
    xtb = np.ascontiguousarray(
        xs[:, :, :, 0:P].transpose(0, 2, 1, 3)).astype(bf)

    def pack8(W):
        return np.ascontiguousarray(
            (W.T * WSCALE).reshape(CT, P, D).transpose(1, 0, 2)).astype(E4)

    wq8, wk8, wv8 = pack8(Wq), pack8(Wk), pack8(Wv)
    wqT = np.ascontiguousarray(Wq.T).astype(bf)
    wkT = np.ascontiguousarray(Wk.T).astype(bf)
    wvT = np.ascontiguousarray(Wv.T * WSCALE).astype(bf)  # 16x domain

    cfs = np.zeros((P, 2 * DT), dtype=f32)
    cfs[:, 0:DT] = bq.astype(f32).reshape(DT, P).T
    cfs[:, DT:2 * DT] = bk.astype(f32).reshape(DT, P).T
    cf32 = np.zeros((P, 3 * D), dtype=f32)
    cf32[:, 0:D] = bq.astype(f32)[None, :]
    cf32[:, D:2 * D] = bk.astype(f32)[None, :]
    cf32[:, 2 * D:3 * D] = bv.astype(f32)[None, :] * WSCALE
    cf16 = np.zeros((P, 2 * P), dtype=bf)
    cf16[:, 0:P] = (np.arange(P)[:, None] <= np.arange(P)[None, :])
    cf16[:, P:2 * P] = np.eye(P)

    pm = np.where(mask[:, 0, :] != 0, 0.0, NEG).astype(f32)  # [B, L]
    pmt = np.ascontiguousarray(
        pm.reshape(B, LT, P).transpose(2, 0, 1))  # [P, B, LT]
    return x8b, xtb, wq8, wk8, wv8, wqT, wkT, wvT, cfs, cf32, cf16, pmt


_NC_CACHE = {}


def kernel(x, Wq, bq, Wk, bk, Wv, bv, mask):
    x = np.asarray(x)
    Wq, bq = np.asarray(Wq), np.asarray(bq)
    Wk, bk = np.asarray(Wk), np.asarray(bk)
    Wv, bv = np.asarray(Wv), np.asarray(bv)
    mask = np.asarray(mask)

    (x8b, xtb, wq8, wk8, wv8, wqT, wkT, wvT, cfs, cf32, cf16, pmt) = _prep_host(
        x, Wq, bq, Wk, bk, Wv, bv, mask)

    if "nc" not in _NC_CACHE:
        _NC_CACHE["nc"] = build_program(NB)
    nc = _NC_CACHE["nc"]

    in_maps = []
    for c in range(N_CORES):
        s = slice(c * NB, (c + 1) * NB)
        in_maps.append({
            "x8b": np.ascontiguousarray(x8b[s]),
            "xtb": np.ascontiguousarray(xtb[s]),
            "wq8": wq8, "wk8": wk8, "wv8": wv8,
            "wqT": wqT, "wkT": wkT, "wvT": wvT,
            "cfs": cfs, "cf32": cf32, "cf16": cf16,
            "pmt": np.ascontiguousarray(pmt[:, s]),
        })

    res = bass_utils.run_bass_kernel_spmd(
        nc, in_maps, core_ids=list(range(N_CORES)),
        trace=bool(int(os.environ.get("KERNEL_TRACE", "0"))),
    )
    if os.environ.get("KERNEL_RESULT_HOOK"):
        _NC_CACHE["last_result"] = res

    return np.concatenate([res.results[c]["out"] for c in range(N_CORES)],
                          axis=0).astype(np.float32)


# revision 24
# speedup vs baseline: 1.1838x; 1.0079x over previous
"""Causal attention (QKV proj + softmax + PV + ReLU) on 8 trn2 NeuronCores.

Sharding: data-parallel over batch B=32 -> 4 batches per core; projection
weights replicated.

Dtypes: everything that can tolerate it runs in fp8-e4m3 with DoubleRow
matmuls (two 128-row contraction slabs per pass -> 2x PE throughput vs
fp16): the Q/K projections, S=K^T.T@Q^T, the V projection, and P@V.
Accumulation is always fp32 in PSUM. Error control:
  - weights are pre-scaled by 16 on the host so their U(-1/32,1/32)
    entries escape e4m3's subnormal range (descale folded into drains; for
    V the 16x rides the value domain and cancels exactly in the rowsum
    normalization, via 16.0-valued `ones`).
  - rows i<128 get a full fp16 path (fp16 projections of q,k,v for l<128,
    fp16 S block, fp16 PV for ib=0): few-term softmax rows pass quant
    noise straight to the output, everything later averages it away.
    Measured rel-err 6.7e-3 vs the 2e-2 gate.
  - the fp16 early-block q,k come from V-style x-stationary N=512 matmuls
    (full PE rate) + 8 PE transposes to d-major, interleaved into the V
    phase so transpose LDWEIGHTS hide under N=512 streams.

Schedule: S(jb) and PV(ib) are interleaved (stagger 2) so PV matmuls keep
the PE busy while ACT drains exps; small constants ride in two packed
DMAs so the weight loads aren't stuck behind seven tiny descriptors; pm
biases for all batches load once. DMA queues are program-ordered per
engine: x prefetch on nc.sync, consts/weights on nc.scalar, stores on
nc.gpsimd. Dummy warmup matmuls pre-warm the PE clock-gate while batch-0
inputs stream in.
"""

import os
from contextlib import ExitStack

import numpy as np
import ml_dtypes

import concourse.tile as tile
from concourse import bacc, mybir
from concourse import bass_utils

F32 = mybir.dt.float32
F16 = mybir.dt.float16
F8 = mybir.dt.float8e4
E4 = ml_dtypes.float8_e4m3
AF = mybir.ActivationFunctionType
DRM = mybir.MatmulPerfMode.DoubleRow

N_CORES = 8
B = 32
L = 1024
C = 1024  # d_model
D = 512
P = 128
NB = B // N_CORES  # batches per core
CT = C // P  # 8 contraction slabs
DT = D // P  # 4 d slabs
LT = L // P  # 8 l/j/i tiles
SCALE = float(D) ** -0.5
NEG = -30000.0
WSCALE = 16.0  # fp8 weight pre-scale (escapes e4m3 subnormals)

# packed fp32 const layout (columns)
_BQ2, _BK2 = 0, DT
_BQB, _BKB, _BVB = 2 * DT, 2 * DT + D, 2 * DT + 2 * D
F32PACK = 2 * DT + 3 * D


def build_program(nb: int = NB):
    """Build the per-core Bass program for nb batches."""
    nc = bacc.Bacc("TRN2", target_bir_lowering=False, debug=False,
                   num_devices=N_CORES)

    x8b = nc.dram_tensor("x8b", [nb, P, CT, L], F8, kind="ExternalInput").ap()
    xtb = nc.dram_tensor("xtb", [nb, P, CT, P], F16, kind="ExternalInput").ap()
    wq8 = nc.dram_tensor("wq8", [P, CT, D], F8, kind="ExternalInput").ap()
    wk8 = nc.dram_tensor("wk8", [P, CT, D], F8, kind="ExternalInput").ap()
    wv8 = nc.dram_tensor("wv8", [P, CT, D], F8, kind="ExternalInput").ap()
    wqT = nc.dram_tensor("wqT", [P, CT, D], F16, kind="ExternalInput").ap()
    wkT = nc.dram_tensor("wkT", [P, CT, D], F16, kind="ExternalInput").ap()
    wvT = nc.dram_tensor("wvT", [P, CT, D], F16, kind="ExternalInput").ap()
    cfs = nc.dram_tensor("cfs", [P, 2 * DT], F32, kind="ExternalInput").ap()
    cf32 = nc.dram_tensor("cf32", [P, 3 * D], F32, kind="ExternalInput").ap()
    cf16 = nc.dram_tensor("cf16", [P, 2 * P], F16, kind="ExternalInput").ap()
    pmt = nc.dram_tensor("pmt", [P, nb, LT], F32, kind="ExternalInput").ap()
    out = nc.dram_tensor("out", [nb, L, D], F16, kind="ExternalOutput").ap()

    with tile.TileContext(nc) as tc, ExitStack() as ctx:
        const = ctx.enter_context(tc.tile_pool(name="const", bufs=1))
        x8_pool = ctx.enter_context(tc.tile_pool(name="x8", bufs=2))
        xt_pool = ctx.enter_context(tc.tile_pool(name="xt", bufs=2))
        qk_pool = ctx.enter_context(tc.tile_pool(name="qk", bufs=2))
        blk_pool = ctx.enter_context(tc.tile_pool(name="blk", bufs=2))
        v_pool = ctx.enter_context(tc.tile_pool(name="v", bufs=2))
        pt_pool = ctx.enter_context(tc.tile_pool(name="pt", bufs=2))
        o_pool = ctx.enter_context(tc.tile_pool(name="o", bufs=3))
        sm_pool = ctx.enter_context(tc.tile_pool(name="sm", bufs=4))
        mm_ps = ctx.enter_context(tc.tile_pool(name="mmps", bufs=4, space="PSUM"))
        o_ps = ctx.enter_context(tc.tile_pool(name="ops", bufs=2, space="PSUM"))
        r_ps = ctx.enter_context(tc.tile_pool(name="rps", bufs=2, space="PSUM"))

        # --- constants on the scalar HWDGE queue (sync is for x prefetch).
        # wq8 first (it gates the very first matmul), tiny packs next. ---
        wq_sb = const.tile([P, CT, D], F8)
        nc.scalar.dma_start(wq_sb[:], wq8[:])
        cfs_sb = const.tile([P, 2 * DT], F32)
        nc.scalar.dma_start(cfs_sb[:], cfs[:])
        wk_sb = const.tile([P, CT, D], F8)
        nc.scalar.dma_start(wk_sb[:], wk8[:])
        wv8_sb = const.tile([P, CT, D], F8)
        nc.scalar.dma_start(wv8_sb[:], wv8[:])
        pm_all = const.tile([P, nb, LT], F32)
        nc.scalar.dma_start(pm_all[:], pmt[:])
        cf32_sb = const.tile([P, 3 * D], F32)
        nc.scalar.dma_start(cf32_sb[:], cf32[:])
        cf16_sb = const.tile([P, 2 * P], F16)
        nc.scalar.dma_start(cf16_sb[:], cf16[:])
        wq16_sb = const.tile([P, CT, D], F16)
        nc.scalar.dma_start(wq16_sb[:], wqT[:])
        wk16_sb = const.tile([P, CT, D], F16)
        nc.scalar.dma_start(wk16_sb[:], wkT[:])
        wv16_sb = const.tile([P, CT, D], F16)
        nc.scalar.dma_start(wv16_sb[:], wvT[:])

        bq_sb = cfs_sb[:, 0:DT]
        bk_sb = cfs_sb[:, DT:2 * DT]
        bqb_sb = cf32_sb[:, 0:D]
        bkb_sb = cf32_sb[:, D:2 * D]
        bvb_sb = cf32_sb[:, 2 * D:3 * D]  # 16*bv broadcast
        tri_sb = cf16_sb[:, 0:P]
        idm_sb = cf16_sb[:, P:2 * P]

        ones16_sb = const.tile([P, 1], F16)
        nc.vector.memset(ones16_sb[:], WSCALE)
        ones8_sb = const.tile([P, 2, 16], F8)
        nc.vector.memset(ones8_sb[:], WSCALE)

        # PE warmup: dummy matmuls with no input deps keep the PE busy while
        # wq8/x8 stream in, so the HAM clock-gate is at 2.4 GHz when the
        # real stream starts.
        warm_sb = const.tile([P, 512], F16)
        nc.vector.memset(warm_sb[:], 0.0)
        for w in range(9):
            wps = mm_ps.tile([P, 512], F32, tag="ps", name=f"warm{w}")
            nc.tensor.matmul(wps[:], warm_sb[:, 0:P], warm_sb[:],
                             start=True, stop=True)

        for b in range(nb):
            # --- X fp8 slab-major [128, CT, L] for Q/K/V ---
            x8 = x8_pool.tile([P, CT, L], F8, tag="x8", name=f"x8_{b}")
            if b == 0:
                # first batch: l<512 halves first so Q lc=0 starts earlier
                nc.sync.dma_start(x8[:, :, 0:512], x8b[b][:, :, 0:512])
                nc.sync.dma_start(x8[:, :, 512:L], x8b[b][:, :, 512:L])
            else:
                nc.sync.dma_start(x8[:], x8b[b])
            # --- X^T fp16 l<128 slab-major tile for the early block ---
            xt16 = xt_pool.tile([P, CT, P], F16, tag="xt", name=f"xt_{b}")
            nc.sync.dma_start(xt16[:], xtb[b])

            # --- Q^T, K^T: fp8 [128, DT, L] tiles; DoubleRow over 2-slab
            # pairs; descale+bias folded into the DVE drain ---
            qt = qk_pool.tile([P, DT, L], F8, tag="qt", name=f"qt_{b}")
            kt = qk_pool.tile([P, DT, L], F8, tag="kt", name=f"kt_{b}")
            for name, w_sb, b_sb, dst in (("q", wq_sb, bq_sb, qt),
                                          ("k", wk_sb, bk_sb, kt)):
                if b == 0 and name == "q":
                    # lc-outer so all lc=0 groups run on the early halves
                    order = [(dt, lc) for lc in range(L // 512)
                             for dt in range(DT)]
                else:
                    order = [(dt, lc) for dt in range(DT)
                             for lc in range(L // 512)]
                for dt, lc in order:
                    ps = mm_ps.tile([P, 512], F32, tag="ps",
                                    name=f"{name}ps{dt}_{lc}_{b}")
                    for s in range(CT // 2):
                        nc.tensor.matmul(
                            ps[:],
                            w_sb[:, 2 * s:2 * s + 2, dt * P:(dt + 1) * P],
                            x8[:, 2 * s:2 * s + 2, lc * 512:(lc + 1) * 512],
                            start=(s == 0), stop=(s == CT // 2 - 1),
                            perf_mode=DRM)
                    nc.vector.tensor_scalar(
                        dst[:, dt, lc * 512:(lc + 1) * 512], ps[:],
                        1.0 / WSCALE, b_sb[:, dt:dt + 1],
                        mybir.AluOpType.mult, mybir.AluOpType.add)

            # --- phases: V fp8 pair tiles [128l, 2, 512d] (DoubleRow, 16x
            # value domain); early-block q,k [l<128, d] fp16 (x-stationary,
            # N=512 — rows i<128 need an accurate fp16 path, fp8 logit noise
            # doesn't average out in few-term softmaxes) + 8 PE transposes
            # to d-major; v0 as fp16 for the ib=0 rim. For b=0, V8 runs
            # first (wv8 lands ~7us; the fat fp16 weights land ~19us). ---
            blkq = blk_pool.tile([P, D], F16, tag="blkq", name=f"blkq_{b}")
            blkk = blk_pool.tile([P, D], F16, tag="blkk", name=f"blkk_{b}")
            qt16 = blk_pool.tile([P, DT, P], F16, tag="qt16", name=f"qt16_{b}")
            kt16 = blk_pool.tile([P, DT, P], F16, tag="kt16", name=f"kt16_{b}")
            v8p = [v_pool.tile([P, 2, D], F8, tag=f"v8p{t}", name=f"v8p{t}_{b}")
                   for t in range(LT // 2)]
            v016 = v_pool.tile([P, D], F16, tag="v016", name=f"v016_{b}")
            tponder = [(blkq, qt16, ds) for ds in range(DT)] + \
                      [(blkk, kt16, ds) for ds in range(DT)]

            def emit_blk():
                for w16_sb, bb_sb, blk in ((wq16_sb, bqb_sb, blkq),
                                           (wk16_sb, bkb_sb, blkk)):
                    ps = mm_ps.tile([P, D], F32, tag="ps", name=f"blkps_{b}")
                    for ct in range(CT):
                        nc.tensor.matmul(ps[:], xt16[:, ct, :],
                                         w16_sb[:, ct, :],
                                         start=(ct == 0), stop=(ct == CT - 1))
                    nc.vector.tensor_add(blk[:], ps[:], bb_sb)

            def emit_v8(lt, transp):
                ps = mm_ps.tile([P, D], F32, tag="ps", name=f"vps{lt}_{b}")
                for s in range(CT // 2):
                    nc.tensor.matmul(
                        ps[:],
                        x8[:, 2 * s:2 * s + 2, lt * P:(lt + 1) * P],
                        wv8_sb[:, 2 * s:2 * s + 2, :],
                        start=(s == 0), stop=(s == CT // 2 - 1),
                        perf_mode=DRM)
                nc.vector.tensor_add(v8p[lt // 2][:, lt % 2, :], ps[:],
                                     bvb_sb)
                if transp and lt < len(tponder):
                    emit_transpose(lt)

            def emit_transpose(k):
                blk, dstT, ds = tponder[k]
                tps = mm_ps.tile([P, P], F16, tag="ps", name=f"tps{k}_{b}")
                nc.tensor.transpose(tps[:], blk[:, ds * P:(ds + 1) * P],
                                    idm_sb)
                nc.vector.tensor_copy(dstT[:, ds, :], tps[:])

            def emit_v016():
                ps = mm_ps.tile([P, D], F32, tag="ps", name=f"v016ps_{b}")
                for ct in range(CT):
                    nc.tensor.matmul(ps[:], xt16[:, ct, :], wv16_sb[:, ct, :],
                                     start=(ct == 0), stop=(ct == CT - 1))
                nc.vector.tensor_add(v016[:], ps[:], bvb_sb)


            # --- S^T tiles + exp -> P^T fp8 pair tiles (causal: only
            # i >= j0 computed), interleaved with PV (stagger 2) so PV
            # matmuls keep the PE busy while ACT drains exps ---
            ptp = [pt_pool.tile([P, 2, L], F8, tag=f"ptp{t}",
                                name=f"ptp{t}_{b}")
                   for t in range(LT // 2)]
            pt0h = pt_pool.tile([P, P], F16, tag="pt0h", name=f"pt0h_{b}")

            def emit_s(jb):
                j0 = jb * P
                tIdx, slab = jb // 2, jb % 2
                i0 = j0
                if jb == 0:
                    # rows i<128: accurate fp16 path from qt16/kt16
                    ps = mm_ps.tile([P, P], F32, tag="ps", name=f"sps0h_{b}")
                    for dt in range(DT):
                        nc.tensor.matmul(ps[:], kt16[:, dt, :], qt16[:, dt, :],
                                         start=(dt == 0), stop=(dt == DT - 1))
                    nc.scalar.activation(pt0h[:], ps[:], AF.Exp,
                                         bias=pm_all[:, b, 0:1], scale=SCALE)
                    nc.vector.tensor_mul(pt0h[:], pt0h[:], tri_sb)
                    i0 = P
                while i0 < L:
                    n = min((i0 // 512 + 1) * 512, L) - i0
                    ps = mm_ps.tile([P, n], F32, tag="ps",
                                    name=f"sps{jb}_{i0}_{b}")
                    for s in range(DT // 2):
                        nc.tensor.matmul(
                            ps[:],
                            kt[:, 2 * s:2 * s + 2, j0:j0 + P],
                            qt[:, 2 * s:2 * s + 2, i0:i0 + n],
                            start=(s == 0), stop=(s == DT // 2 - 1),
                            perf_mode=DRM)
                    nc.scalar.activation(ptp[tIdx][:, slab, i0:i0 + n], ps[:],
                                         AF.Exp, bias=pm_all[:, b, jb:jb + 1],
                                         scale=SCALE)
                    i0 += n
                if jb > 0:
                    # mask the diagonal tile: keep j<=i
                    nc.vector.tensor_mul(ptp[tIdx][:, slab, j0:j0 + P],
                                         ptp[tIdx][:, slab, j0:j0 + P],
                                         tri_sb)

            def emit_pv(ib):
                i0 = ib * P
                ops = o_ps.tile([P, D], F32, tag="op", name=f"ops{ib}_{b}")
                rps = r_ps.tile([P, 1], F32, tag="rp", name=f"rps{ib}_{b}")
                if ib == 0:
                    nc.tensor.matmul(ops[:], pt0h[:], v016[:],
                                     start=True, stop=True)
                    nc.tensor.matmul(rps[:], pt0h[:], ones16_sb[:],
                                     start=True, stop=True)
                else:
                    # fp8 DoubleRow over jb pairs; odd tail as plain fp8.
                    # rowsum piggybacks with 16.0-ones: out = (sum p*16v)
                    # / (16*sum p) — the 16x V domain cancels exactly.
                    npair = (ib + 1) // 2
                    leftover = (ib + 1) % 2
                    last = npair + leftover - 1
                    for t in range(npair):
                        st, sp = (t == 0), (t == last and not leftover)
                        pT = ptp[t][:, :, i0:i0 + P]
                        nc.tensor.matmul(ops[:], pT, v8p[t][:],
                                         start=st, stop=sp, perf_mode=DRM)
                        nc.tensor.matmul(rps[:], pT, ones8_sb[:, :, 0:1],
                                         start=st, stop=sp, perf_mode=DRM)
                    if leftover:
                        t = npair
                        pT = ptp[t][:, 0, i0:i0 + P]
                        nc.tensor.matmul(ops[:], pT, v8p[t][:, 0, :],
                                         start=(npair == 0), stop=True)
                        nc.tensor.matmul(rps[:], pT, ones8_sb[:, 0, 0:1],
                                         start=(npair == 0), stop=True)
                rec = sm_pool.tile([P, 1], F32, tag="rec", name=f"rec{ib}_{b}")
                nc.vector.reciprocal(rec[:], rps[:])
                o_sb = o_pool.tile([P, D], F16, tag="ot", name=f"o{ib}_{b}")
                # relu(O'/rowsum) on DVE: (in0 * rec) max 0 — keeps ACT free
                # for the exps, whose latency gates S-phase PSUM slot reuse
                nc.vector.tensor_scalar(o_sb[:], ops[:], rec[:], 0.0,
                                        mybir.AluOpType.mult,
                                        mybir.AluOpType.max)
                # SWDGE so stores never head-of-line-block the x prefetch;
                # last batch has no prefetch left, so use the faster HWDGE
                if b == nb - 1:
                    nc.sync.dma_start(out[b, i0:i0 + P, :], o_sb[:])
                else:
                    nc.gpsimd.dma_start(out[b, i0:i0 + P, :], o_sb[:])

            emit_blk()
            for lt in range(LT):
                emit_v8(lt, transp=True)
            emit_v016()
            STAG = 2
            for jb in range(LT):
                emit_s(jb)
                if jb >= STAG:
                    emit_pv(jb - STAG)
            for ib in range(LT - STAG, LT):
                emit_pv(ib)

    nc.compile()
    return nc


def _prep_host(x, Wq, bq, Wk, bk, Wv, bv, mask):
    bf = np.float16
    f32 = np.float32
    xT = np.ascontiguousarray(x.transpose(0, 2, 1))  # [B, C, L]
    xs = xT.reshape(B, CT, P, L)
    # x fp8 slab-major [B, 128, CT, L] for the DoubleRow paths
    x8b = np.ascontiguousarray(xs.transpose(0, 2, 1, 3)).astype(E4)
    # x fp16 l<128 slab-major [B, 128, CT, 128] for the early block
    xtb = np.ascontiguousarray(
        xs[:, :, :, 0:P].transpose(0, 2, 1, 3)).astype(bf)

    def pack8(W):
        return np.ascontiguousarray(
            (W.T * WSCALE).reshape(CT, P, D).transpose(1, 0, 2)).astype(E4)

    wq8, wk8, wv8 = pack8(Wq), pack8(Wk), pack8(Wv)
    def pack16(W, s=1.0):
        return np.ascontiguousarray(
            (W.T * s).reshape(CT, P, D).transpose(1, 0, 2)).astype(bf)

    wqT, wkT = pack16(Wq), pack16(Wk)
    wvT = pack16(Wv, WSCALE)  # 16x domain

    cfs = np.zeros((P, 2 * DT), dtype=f32)
    cfs[:, 0:DT] = bq.astype(f32).reshape(DT, P).T
    cfs[:, DT:2 * DT] = bk.astype(f32).reshape(DT, P).T
    cf32 = np.zeros((P, 3 * D), dtype=f32)
    cf32[:, 0:D] = bq.astype(f32)[None, :]
    cf32[:, D:2 * D] = bk.astype(f32)[None, :]
    cf32[:, 2 * D:3 * D] = bv.astype(f32)[None, :] * WSCALE
    cf16 = np.zeros((P, 2 * P), dtype=bf)
    cf16[:, 0:P] = (np.arange(P)[:, None] <= np.arange(P)[None, :])
    cf16[:, P:2 * P] = np.eye(P)

    pm = np.where(mask[:, 0, :] != 0, 0.0, NEG).astype(f32)  # [B, L]
    pmt = np.ascontiguousarray(
        pm.reshape(B, LT, P).transpose(2, 0, 1))  # [P, B, LT]
    return x8b, xtb, wq8, wk8, wv8, wqT, wkT, wvT, cfs, cf32, cf16, pmt


_NC_CACHE = {}


def kernel(x, Wq, bq, Wk, bk, Wv, bv, mask):
    x = np.asarray(x)
    Wq, bq = np.asarray(Wq), np.asarray(bq)
    Wk, bk = np.asarray(Wk), np.asarray(bk)
    Wv, bv = np.asarray(Wv), np.asarray(bv)
    mask = np.asarray(mask)

    (x8b, xtb, wq8, wk8, wv8, wqT, wkT, wvT, cfs, cf32, cf16, pmt) = _prep_host(
        x, Wq, bq, Wk, bk, Wv, bv, mask)

    if "nc" not in _NC_CACHE:
        _NC_CACHE["nc"] = build_program(NB)
    nc = _NC_CACHE["nc"]

    in_maps = []
    for c in range(N_CORES):
        s = slice(c * NB, (c + 1) * NB)
        in_maps.append({
            "x8b": np.ascontiguousarray(x8b[s]),
            "xtb": np.ascontiguousarray(xtb[s]),
            "wq8": wq8, "wk8": wk8, "wv8": wv8,
            "wqT": wqT, "wkT": wkT, "wvT": wvT,
            "cfs": cfs, "cf32": cf32, "cf16": cf16,
            "pmt": np.ascontiguousarray(pmt[:, s]),
        })

    res = bass_utils.run_bass_kernel_spmd(
        nc, in_maps, core_ids=list(range(N_CORES)),
        trace=bool(int(os.environ.get("KERNEL_TRACE", "0"))),
    )
    if os.environ.get("KERNEL_RESULT_HOOK"):
        _NC_CACHE["last_result"] = res

    return np.concatenate([res.results[c]["out"] for c in range(N_CORES)],
                          axis=0).astype(np.float32)


# revision 26
# speedup vs baseline: 1.1953x; 1.0097x over previous
"""Causal attention (QKV proj + softmax + PV + ReLU) on 8 trn2 NeuronCores.

Sharding: data-parallel over batch B=32 -> 4 batches per core; projection
weights replicated.

Dtypes: everything that can tolerate it runs in fp8-e4m3 with DoubleRow
matmuls (two 128-row contraction slabs per pass -> 2x PE throughput vs
fp16): the Q/K projections, S=K^T.T@Q^T, the V projection, and P@V.
Accumulation is always fp32 in PSUM. Error control:
  - weights are pre-scaled by 16 on the host so their U(-1/32,1/32)
    entries escape e4m3's subnormal range (descale folded into drains; for
    V the 16x rides the value domain and cancels exactly in the rowsum
    normalization, via 16.0-valued `ones`).
  - rows i<128 get a full fp16 path (fp16 projections of q,k,v for l<128,
    fp16 S block, fp16 PV for ib=0): few-term softmax rows pass quant
    noise straight to the output, everything later averages it away.
    Measured rel-err 6.7e-3 vs the 2e-2 gate.
  - the fp16 early-block q,k come from V-style x-stationary N=512 matmuls
    (full PE rate) + 8 PE transposes to d-major, interleaved into the V
    phase so transpose LDWEIGHTS hide under N=512 streams.

Schedule: S(jb) and PV(ib) are interleaved (stagger 2) so PV matmuls keep
the PE busy while ACT drains exps; small constants ride in two packed
DMAs so the weight loads aren't stuck behind seven tiny descriptors; pm
biases for all batches load once. DMA queues are program-ordered per
engine: x prefetch on nc.sync, consts/weights on nc.scalar, stores on
nc.gpsimd. Dummy warmup matmuls pre-warm the PE clock-gate while batch-0
inputs stream in.
"""

import os
from contextlib import ExitStack

import numpy as np
import ml_dtypes

import concourse.tile as tile
from concourse import bacc, mybir
from concourse import bass_utils

F32 = mybir.dt.float32
F16 = mybir.dt.float16
F8 = mybir.dt.float8e4
E4 = ml_dtypes.float8_e4m3
AF = mybir.ActivationFunctionType
DRM = mybir.MatmulPerfMode.DoubleRow

N_CORES = 8
B = 32
L = 1024
C = 1024  # d_model
D = 512
P = 128
NB = B // N_CORES  # batches per core
CT = C // P  # 8 contraction slabs
DT = D // P  # 4 d slabs
LT = L // P  # 8 l/j/i tiles
SCALE = float(D) ** -0.5
NEG = -30000.0
WSCALE = 16.0  # fp8 weight pre-scale (escapes e4m3 subnormals)

# packed fp32 const layout (columns)
_BQ2, _BK2 = 0, DT
_BQB, _BKB, _BVB = 2 * DT, 2 * DT + D, 2 * DT + 2 * D
F32PACK = 2 * DT + 3 * D


def build_program(nb: int = NB):
    """Build the per-core Bass program for nb batches."""
    nc = bacc.Bacc("TRN2", target_bir_lowering=False, debug=False,
                   num_devices=N_CORES)

    x8b = nc.dram_tensor("x8b", [nb, P, CT, L], F8, kind="ExternalInput").ap()
    xtb = nc.dram_tensor("xtb", [nb, P, CT, P], F16, kind="ExternalInput").ap()
    wq8 = nc.dram_tensor("wq8", [P, CT, D], F8, kind="ExternalInput").ap()
    wk8 = nc.dram_tensor("wk8", [P, CT, D], F8, kind="ExternalInput").ap()
    wv8 = nc.dram_tensor("wv8", [P, CT, D], F8, kind="ExternalInput").ap()
    wqT = nc.dram_tensor("wqT", [P, CT, D], F16, kind="ExternalInput").ap()
    wkT = nc.dram_tensor("wkT", [P, CT, D], F16, kind="ExternalInput").ap()
    wvT = nc.dram_tensor("wvT", [P, CT, D], F16, kind="ExternalInput").ap()
    cfs = nc.dram_tensor("cfs", [P, 2 * DT], F32, kind="ExternalInput").ap()
    cf32 = nc.dram_tensor("cf32", [P, 3 * D], F32, kind="ExternalInput").ap()
    cf16 = nc.dram_tensor("cf16", [P, 2 * P], F16, kind="ExternalInput").ap()
    pmt = nc.dram_tensor("pmt", [P, nb, LT], F32, kind="ExternalInput").ap()
    out = nc.dram_tensor("out", [nb, L, D], F16, kind="ExternalOutput").ap()

    with tile.TileContext(nc) as tc, ExitStack() as ctx:
        const = ctx.enter_context(tc.tile_pool(name="const", bufs=1))
        x8_pool = ctx.enter_context(tc.tile_pool(name="x8", bufs=2))
        xt_pool = ctx.enter_context(tc.tile_pool(name="xt", bufs=2))
        qk_pool = ctx.enter_context(tc.tile_pool(name="qk", bufs=2))
        blk_pool = ctx.enter_context(tc.tile_pool(name="blk", bufs=2))
        v_pool = ctx.enter_context(tc.tile_pool(name="v", bufs=2))
        pt_pool = ctx.enter_context(tc.tile_pool(name="pt", bufs=2))
        o_pool = ctx.enter_context(tc.tile_pool(name="o", bufs=3))
        sm_pool = ctx.enter_context(tc.tile_pool(name="sm", bufs=4))
        mm_ps = ctx.enter_context(tc.tile_pool(name="mmps", bufs=4, space="PSUM"))
        o_ps = ctx.enter_context(tc.tile_pool(name="ops", bufs=2, space="PSUM"))
        r_ps = ctx.enter_context(tc.tile_pool(name="rps", bufs=2, space="PSUM"))

        # --- constants on the scalar HWDGE queue (sync is for x prefetch).
        # wq8 first (it gates the very first matmul), tiny packs next. ---
        wq_sb = const.tile([P, CT, D], F8)
        nc.scalar.dma_start(wq_sb[:], wq8[:])
        cfs_sb = const.tile([P, 2 * DT], F32)
        nc.scalar.dma_start(cfs_sb[:], cfs[:])
        wk_sb = const.tile([P, CT, D], F8)
        nc.scalar.dma_start(wk_sb[:], wk8[:])
        wv8_sb = const.tile([P, CT, D], F8)
        nc.scalar.dma_start(wv8_sb[:], wv8[:])
        pm_all = const.tile([P, nb, LT], F32)
        nc.scalar.dma_start(pm_all[:], pmt[:])
        cf32_sb = const.tile([P, 3 * D], F32)
        nc.scalar.dma_start(cf32_sb[:], cf32[:])
        cf16_sb = const.tile([P, 2 * P], F16)
        nc.scalar.dma_start(cf16_sb[:], cf16[:])
        wq16_sb = const.tile([P, CT, D], F16)
        nc.scalar.dma_start(wq16_sb[:], wqT[:])
        wk16_sb = const.tile([P, CT, D], F16)
        nc.scalar.dma_start(wk16_sb[:], wkT[:])
        wv16_sb = const.tile([P, CT, D], F16)
        nc.scalar.dma_start(wv16_sb[:], wvT[:])

        bq_sb = cfs_sb[:, 0:DT]
        bk_sb = cfs_sb[:, DT:2 * DT]
        bqb_sb = cf32_sb[:, 0:D]
        bkb_sb = cf32_sb[:, D:2 * D]
        bvb_sb = cf32_sb[:, 2 * D:3 * D]  # 16*bv broadcast
        tri_sb = cf16_sb[:, 0:P]
        idm_sb = cf16_sb[:, P:2 * P]

        ones16_sb = const.tile([P, 1], F16)
        nc.vector.memset(ones16_sb[:], WSCALE)
        ones8_sb = const.tile([P, 2, 16], F8)
        nc.vector.memset(ones8_sb[:], WSCALE)

        # PE warmup: dummy matmuls with no input deps keep the PE busy while
        # wq8/x8 stream in, so the HAM clock-gate is at 2.4 GHz when the
        # real stream starts.
        warm_sb = const.tile([P, 512], F16)
        nc.vector.memset(warm_sb[:], 0.0)
        for w in range(13):
            wps = mm_ps.tile([P, 512], F32, tag="ps", name=f"warm{w}")
            nc.tensor.matmul(wps[:], warm_sb[:, 0:P], warm_sb[:],
                             start=True, stop=True)

        for b in range(nb):
            # --- X fp8 slab-major [128, CT, L] for Q/K/V ---
            x8 = x8_pool.tile([P, CT, L], F8, tag="x8", name=f"x8_{b}")
            if b == 0:
                # first batch: l<512 halves first so Q lc=0 starts earlier
                nc.sync.dma_start(x8[:, :, 0:512], x8b[b][:, :, 0:512])
                nc.sync.dma_start(x8[:, :, 512:L], x8b[b][:, :, 512:L])
            else:
                nc.sync.dma_start(x8[:], x8b[b])
            # --- X^T fp16 l<128 slab-major tile for the early block ---
            xt16 = xt_pool.tile([P, CT, P], F16, tag="xt", name=f"xt_{b}")
            nc.sync.dma_start(xt16[:], xtb[b])

            # --- Q^T, K^T: fp8 [128, DT, L] tiles; DoubleRow over 2-slab
            # pairs; descale+bias folded into the DVE drain ---
            qt = qk_pool.tile([P, DT, L], F8, tag="qt", name=f"qt_{b}")
            kt = qk_pool.tile([P, DT, L], F8, tag="kt", name=f"kt_{b}")
            for name, w_sb, b_sb, dst in (("q", wq_sb, bq_sb, qt),
                                          ("k", wk_sb, bk_sb, kt)):
                if b == 0 and name == "q":
                    # lc-outer so all lc=0 groups run on the early halves
                    order = [(dt, lc) for lc in range(L // 512)
                             for dt in range(DT)]
                else:
                    order = [(dt, lc) for dt in range(DT)
                             for lc in range(L // 512)]
                for dt, lc in order:
                    ps = mm_ps.tile([P, 512], F32, tag="ps",
                                    name=f"{name}ps{dt}_{lc}_{b}")
                    for s in range(CT // 2):
                        nc.tensor.matmul(
                            ps[:],
                            w_sb[:, 2 * s:2 * s + 2, dt * P:(dt + 1) * P],
                            x8[:, 2 * s:2 * s + 2, lc * 512:(lc + 1) * 512],
                            start=(s == 0), stop=(s == CT // 2 - 1),
                            perf_mode=DRM)
                    nc.vector.tensor_scalar(
                        dst[:, dt, lc * 512:(lc + 1) * 512], ps[:],
                        1.0 / WSCALE, b_sb[:, dt:dt + 1],
                        mybir.AluOpType.mult, mybir.AluOpType.add)

            # --- phases: V fp8 pair tiles [128l, 2, 512d] (DoubleRow, 16x
            # value domain); early-block q,k [l<128, d] fp16 (x-stationary,
            # N=512 — rows i<128 need an accurate fp16 path, fp8 logit noise
            # doesn't average out in few-term softmaxes) + 8 PE transposes
            # to d-major; v0 as fp16 for the ib=0 rim. For b=0, V8 runs
            # first (wv8 lands ~7us; the fat fp16 weights land ~19us). ---
            blkq = blk_pool.tile([P, D], F16, tag="blkq", name=f"blkq_{b}")
            blkk = blk_pool.tile([P, D], F16, tag="blkk", name=f"blkk_{b}")
            qt16 = blk_pool.tile([P, DT, P], F16, tag="qt16", name=f"qt16_{b}")
            kt16 = blk_pool.tile([P, DT, P], F16, tag="kt16", name=f"kt16_{b}")
            v8p = [v_pool.tile([P, 2, D], F8, tag=f"v8p{t}", name=f"v8p{t}_{b}")
                   for t in range(LT // 2)]
            v016 = v_pool.tile([P, D], F16, tag="v016", name=f"v016_{b}")
            tponder = [(blkq, qt16, ds) for ds in range(DT)] + \
                      [(blkk, kt16, ds) for ds in range(DT)]

            def emit_blk():
                for w16_sb, bb_sb, blk in ((wq16_sb, bqb_sb, blkq),
                                           (wk16_sb, bkb_sb, blkk)):
                    ps = mm_ps.tile([P, D], F32, tag="ps", name=f"blkps_{b}")
                    for ct in range(CT):
                        nc.tensor.matmul(ps[:], xt16[:, ct, :],
                                         w16_sb[:, ct, :],
                                         start=(ct == 0), stop=(ct == CT - 1))
                    nc.vector.tensor_add(blk[:], ps[:], bb_sb)

            def emit_v8(lt, transp):
                ps = mm_ps.tile([P, D], F32, tag="ps", name=f"vps{lt}_{b}")
                for s in range(CT // 2):
                    nc.tensor.matmul(
                        ps[:],
                        x8[:, 2 * s:2 * s + 2, lt * P:(lt + 1) * P],
                        wv8_sb[:, 2 * s:2 * s + 2, :],
                        start=(s == 0), stop=(s == CT // 2 - 1),
                        perf_mode=DRM)
                nc.vector.tensor_add(v8p[lt // 2][:, lt % 2, :], ps[:],
                                     bvb_sb)
                if transp and lt < len(tponder):
                    emit_transpose(lt)

            def emit_transpose(k):
                blk, dstT, ds = tponder[k]
                tps = mm_ps.tile([P, P], F16, tag="ps", name=f"tps{k}_{b}")
                nc.tensor.transpose(tps[:], blk[:, ds * P:(ds + 1) * P],
                                    idm_sb)
                nc.vector.tensor_copy(dstT[:, ds, :], tps[:])

            def emit_v016():
                ps = mm_ps.tile([P, D], F32, tag="ps", name=f"v016ps_{b}")
                for ct in range(CT):
                    nc.tensor.matmul(ps[:], xt16[:, ct, :], wv16_sb[:, ct, :],
                                     start=(ct == 0), stop=(ct == CT - 1))
                nc.vector.tensor_add(v016[:], ps[:], bvb_sb)


            # --- S^T tiles + exp -> P^T fp8 pair tiles (causal: only
            # i >= j0 computed), interleaved with PV (stagger 2) so PV
            # matmuls keep the PE busy while ACT drains exps ---
            ptp = [pt_pool.tile([P, 2, L], F8, tag=f"ptp{t}",
                                name=f"ptp{t}_{b}")
                   for t in range(LT // 2)]
            pt0h = pt_pool.tile([P, P], F16, tag="pt0h", name=f"pt0h_{b}")

            def emit_s(jb):
                j0 = jb * P
                tIdx, slab = jb // 2, jb % 2
                i0 = j0
                if jb == 0:
                    # rows i<128: accurate fp16 path from qt16/kt16
                    ps = mm_ps.tile([P, P], F32, tag="ps", name=f"sps0h_{b}")
                    for dt in range(DT):
                        nc.tensor.matmul(ps[:], kt16[:, dt, :], qt16[:, dt, :],
                                         start=(dt == 0), stop=(dt == DT - 1))
                    nc.scalar.activation(pt0h[:], ps[:], AF.Exp,
                                         bias=pm_all[:, b, 0:1], scale=SCALE)
                    nc.vector.tensor_mul(pt0h[:], pt0h[:], tri_sb)
                    i0 = P
                while i0 < L:
                    n = min((i0 // 512 + 1) * 512, L) - i0
                    ps = mm_ps.tile([P, n], F32, tag="ps",
                                    name=f"sps{jb}_{i0}_{b}")
                    for s in range(DT // 2):
                        nc.tensor.matmul(
                            ps[:],
                            kt[:, 2 * s:2 * s + 2, j0:j0 + P],
                            qt[:, 2 * s:2 * s + 2, i0:i0 + n],
                            start=(s == 0), stop=(s == DT // 2 - 1),
                            perf_mode=DRM)
                    nc.scalar.activation(ptp[tIdx][:, slab, i0:i0 + n], ps[:],
                                         AF.Exp, bias=pm_all[:, b, jb:jb + 1],
                                         scale=SCALE)
                    i0 += n
                if jb > 0:
                    # mask the diagonal tile: keep j<=i
                    nc.vector.tensor_mul(ptp[tIdx][:, slab, j0:j0 + P],
                                         ptp[tIdx][:, slab, j0:j0 + P],
                                         tri_sb)

            pv_ps = {}

            def emit_pv(ib, t_hi=None, finish=True):
                i0 = ib * P
                if ib in pv_ps:
                    ops, rps, k = pv_ps.pop(ib)
                else:
                    ops = o_ps.tile([P, D], F32, tag="op", name=f"ops{ib}_{b}")
                    rps = r_ps.tile([P, 1], F32, tag="rp", name=f"rps{ib}_{b}")
                    k = 0
                if ib == 0:
                    nc.tensor.matmul(ops[:], pt0h[:], v016[:],
                                     start=True, stop=True)
                    nc.tensor.matmul(rps[:], pt0h[:], ones16_sb[:],
                                     start=True, stop=True)
                else:
                    # fp8 DoubleRow over jb pairs; odd tail as plain fp8.
                    # rowsum piggybacks with 16.0-ones: out = (sum p*16v)
                    # / (16*sum p) — the 16x V domain cancels exactly.
                    # t_hi/finish allow splitting the accumulation so the
                    # last batch's final PV has a short post-S(7) chain.
                    npair = (ib + 1) // 2
                    leftover = (ib + 1) % 2
                    last = npair + leftover - 1
                    hi = npair if t_hi is None else min(t_hi, npair)
                    for t in range(k, hi):
                        st = (t == 0)
                        sp = finish and (t == last and not leftover)
                        pT = ptp[t][:, :, i0:i0 + P]
                        nc.tensor.matmul(ops[:], pT, v8p[t][:],
                                         start=st, stop=sp, perf_mode=DRM)
                        nc.tensor.matmul(rps[:], pT, ones8_sb[:, :, 0:1],
                                         start=st, stop=sp, perf_mode=DRM)
                        k = t + 1
                    if finish and leftover and k == npair:
                        t = npair
                        pT = ptp[t][:, 0, i0:i0 + P]
                        nc.tensor.matmul(ops[:], pT, v8p[t][:, 0, :],
                                         start=(npair == 0), stop=True)
                        nc.tensor.matmul(rps[:], pT, ones8_sb[:, 0, 0:1],
                                         start=(npair == 0), stop=True)
                if not finish:
                    pv_ps[ib] = (ops, rps, k)
                    return
                rec = sm_pool.tile([P, 1], F32, tag="rec", name=f"rec{ib}_{b}")
                nc.vector.reciprocal(rec[:], rps[:])
                o_sb = o_pool.tile([P, D], F16, tag="ot", name=f"o{ib}_{b}")
                # relu(O'/rowsum) on DVE: (in0 * rec) max 0 — keeps ACT free
                # for the exps, whose latency gates S-phase PSUM slot reuse
                nc.vector.tensor_scalar(o_sb[:], ops[:], rec[:], 0.0,
                                        mybir.AluOpType.mult,
                                        mybir.AluOpType.max)
                # SWDGE so stores never head-of-line-block the x prefetch;
                # last batch has no prefetch left, so use the faster HWDGE
                if b == nb - 1:
                    nc.sync.dma_start(out[b, i0:i0 + P, :], o_sb[:])
                else:
                    nc.gpsimd.dma_start(out[b, i0:i0 + P, :], o_sb[:])

            emit_blk()
            if b > 0:
                # v016 before the v8 groups: its "ps"-slot drain (the one
                # the first S chunk's psum waits on, 4 allocations back)
                # clears while the v8 groups stream
                emit_v016()
            for lt in range(LT):
                emit_v8(lt, transp=True)
            if b == 0:
                emit_v016()
            if b == nb - 1:
                # last batch: stagger 1 and pre-accumulate PV(7)'s first 3
                # DoubleRow pairs behind S(6), so only one pair + the
                # normalize/store chain remains after the final exp
                for jb in range(LT):
                    emit_s(jb)
                    if jb >= 1:
                        emit_pv(jb - 1)
                    if jb == 6:
                        emit_pv(7, t_hi=3, finish=False)
                emit_pv(7)
            else:
                STAG = 2
                for jb in range(LT):
                    emit_s(jb)
                    if jb >= STAG:
                        emit_pv(jb - STAG)
                for ib in range(LT - STAG, LT):
                    emit_pv(ib)

    nc.compile()
    return nc


def _prep_host(x, Wq, bq, Wk, bk, Wv, bv, mask):
    bf = np.float16
    f32 = np.float32
    xT = np.ascontiguousarray(x.transpose(0, 2, 1))  # [B, C, L]
    xs = xT.reshape(B, CT, P, L)
    # x fp8 slab-major [B, 128, CT, L] for the DoubleRow paths
    x8b = np.ascontiguousarray(xs.transpose(0, 2, 1, 3)).astype(E4)
    # x fp16 l<128 slab-major [B, 128, CT, 128] for the early block
    xtb = np.ascontiguousarray(
        xs[:, :, :, 0:P].transpose(0, 2, 1, 3)).astype(bf)

    def pack8(W):
        return np.ascontiguousarray(
            (W.T * WSCALE).reshape(CT, P, D).transpose(1, 0, 2)).astype(E4)

    wq8, wk8, wv8 = pack8(Wq), pack8(Wk), pack8(Wv)
    def pack16(W, s=1.0):
        return np.ascontiguousarray(
            (W.T * s).reshape(CT, P, D).transpose(1, 0, 2)).astype(bf)

    wqT, wkT = pack16(Wq), pack16(Wk)
    wvT = pack16(Wv, WSCALE)  # 16x domain

    cfs = np.zeros((P, 2 * DT), dtype=f32)
    cfs[:, 0:DT] = bq.astype(f32).reshape(DT, P).T
    cfs[:, DT:2 * DT] = bk.astype(f32).reshape(DT, P).T
    cf32 = np.zeros((P, 3 * D), dtype=f32)
    cf32[:, 0:D] = bq.astype(f32)[None, :]
    cf32[:, D:2 * D] = bk.astype(f32)[None, :]
    cf32[:, 2 * D:3 * D] = bv.astype(f32)[None, :] * WSCALE
    cf16 = np.zeros((P, 2 * P), dtype=bf)
    cf16[:, 0:P] = (np.arange(P)[:, None] <= np.arange(P)[None, :])
    cf16[:, P:2 * P] = np.eye(P)

    pm = np.where(mask[:, 0, :] != 0, 0.0, NEG).astype(f32)  # [B, L]
    pmt = np.ascontiguousarray(
        pm.reshape(B, LT, P).transpose(2, 0, 1))  # [P, B, LT]
    return x8b, xtb, wq8, wk8, wv8, wqT, wkT, wvT, cfs, cf32, cf16, pmt


_NC_CACHE = {}


def kernel(x, Wq, bq, Wk, bk, Wv, bv, mask):
    x = np.asarray(x)
    Wq, bq = np.asarray(Wq), np.asarray(bq)
    Wk, bk = np.asarray(Wk), np.asarray(bk)
    Wv, bv = np.asarray(Wv), np.asarray(bv)
    mask = np.asarray(mask)

    (x8b, xtb, wq8, wk8, wv8, wqT, wkT, wvT, cfs, cf32, cf16, pmt) = _prep_host(
        x, Wq, bq, Wk, bk, Wv, bv, mask)

    if "nc" not in _NC_CACHE:
        _NC_CACHE["nc"] = build_program(NB)
    nc = _NC_CACHE["nc"]

    in_maps = []
    for c in range(N_CORES):
        s = slice(c * NB, (c + 1) * NB)
        in_maps.append({
            "x8b": np.ascontiguousarray(x8b[s]),
            "xtb": np.ascontiguousarray(xtb[s]),
            "wq8": wq8, "wk8": wk8, "wv8": wv8,
            "wqT": wqT, "wkT": wkT, "wvT": wvT,
            "cfs": cfs, "cf32": cf32, "cf16": cf16,
            "pmt": np.ascontiguousarray(pmt[:, s]),
        })

    res = bass_utils.run_bass_kernel_spmd(
        nc, in_maps, core_ids=list(range(N_CORES)),
        trace=bool(int(os.environ.get("KERNEL_TRACE", "0"))),
    )
    if os.environ.get("KERNEL_RESULT_HOOK"):
        _NC_CACHE["last_result"] = res

    return np.concatenate([res.results[c]["out"] for c in range(N_CORES)],
                          axis=0).astype(np.float32)
